# revision 33
# baseline (speedup 1.0000x reference)
"""BottomUpHTMM upward pass, tuned for the axon-TRN2 real cost profile:
PE matmuls ~free; DVE ~5us/op; Act ~18us/op; Pool ~36us/op; DMA ~43us/instr.
=> 2 DMAs total (one packed input, one packed output), all vector work on
DVE, zero Act/Pool ops, log-likelihood logs on host from shipped raw nu.

Device per core (quarter-tree): level 6->5 via one-hot(symbol) matmuls
(T6 tables), bl = tb*bx, nu selector matmuls (4 chunks per PSUM bank),
reciprocal + positioned-identity broadcast matmul, normalized b5; level
5->4 W matmuls -> unnormalized bl4.  Ships bl4 + raw nu banks (f32).
Host (f64): leaf-ll histogram, ln(nu) sums, depth-4 normalization,
depths 3..0 + root combine.
Partition packing p = i*16 + g.
"""
import sys

import numpy as np

if '/opt/trn_rl_repo' not in sys.path:
    sys.path.insert(0, '/opt/trn_rl_repo')

import ml_dtypes

BF16 = ml_dtypes.bfloat16

K, DEPTH, NTREE, C, MSYM, NGEN = 8, 6, 2, 8, 32, 16
STARTS = [(K ** d - 1) // (K - 1) for d in range(DEPTH + 2)]
NT = STARTS[DEPTH + 1]
CG = C * NGEN                   # 128
NQ = 4
LEAVES_Q = (K ** DEPTH) // NQ   # 65536 leaves per core
NP5 = LEAVES_Q // K             # 8192 depth-5 parents per core
NP4 = NP5 // K                  # 1024 depth-4 parents per core
NBX = NP5 + NP4                 # 9216 interior symbol-gather columns
NCHUNK = 512

# packed input layout (all bf16): [tabs | svh | bxh | xs-broadcast]
TABW = CG + CG + CG + 16 + 1024          # 1424: T6a|T6b|E16x4|selt|Wt
OFF_T6A, OFF_T6B, OFF_E, OFF_SEL, OFF_W = 0, CG, 2 * CG, 3 * CG, 3 * CG + 16
OFF_SV = TABW                            # 2 comparator columns
OFF_BX = TABW + 2                        # 1426
OFF_XS = OFF_BX + NBX                    # 10642
INW = OFF_XS + NP5                       # 18834
# packed output layout (f32): [bl4 (1024) | nu banks (4 x 512)]
ONW = NP4 + 4 * NCHUNK                   # 3072


def _softmax64(x, axis):
    x = np.asarray(x, np.float64)
    e = np.exp(x - x.max(axis=axis, keepdims=True))
    return e / e.sum(axis=axis, keepdims=True)


def _build_tables(A, B, Pi, SP):
    smA = _softmax64(A, 0)
    smB = _softmax64(B, 1)
    smPi = _softmax64(Pi, 0)
    smSP = _softmax64(SP, 0)
    Mmat = smSP[:, None, None, :] * np.transpose(smA, (2, 0, 1, 3))  # [l,i,j,g]
    pb = smPi[:, :, None, :] * smB[:, None, :, :]
    nuL = pb.sum(0)
    betaLeaf = pb / nuL[None]
    llLeaf = np.log(nuL)
    T6 = np.einsum('lijg,jlsg->lsig', Mmat, betaLeaf)
    T6f = T6.reshape(K * MSYM, CG)
    p = np.arange(CG)
    idxA = (p // 16) * MSYM + (p % 16)
    idxB = (p // 16) * MSYM + 16 + (p % 16)
    Wl = np.zeros((K, CG, CG))
    ii = np.arange(C)
    for l in range(K):
        for g in range(NGEN):
            Wl[l, ii[:, None] * NGEN + g, ii[None, :] * NGEN + g] = Mmat[l, :, :, g].T
    BT = np.transpose(smB, (1, 0, 2)).reshape(MSYM, CG)
    E16 = (p[None, :] % NGEN == np.arange(NGEN)[:, None]).astype(np.float64)
    E16x4 = np.concatenate(
        [np.vstack([E16, np.zeros((16, CG))]) for _ in range(4)], axis=0)
    selt = (p[:, None] % NGEN == np.arange(NGEN)[None, :]).astype(np.float64)
    Wt = np.concatenate([Wl[l] for l in range(K)], axis=1)
    tabs = np.concatenate([T6f[idxA], T6f[idxB], E16x4, selt, Wt], axis=1)  # [128,1424]
    svh = np.stack([p % 16, p % 16 + 16], axis=1)                           # [128,2]
    host = {'Mmat': Mmat, 'smB': smB, 'BT': BT, 'llLeaf': llLeaf,
            'tabsv': np.concatenate([tabs, svh], axis=1).astype(BF16)}      # [128,1426]
    return host


def _build_bass(n_reps=1):
    import concourse.bass as bass
    import concourse.bacc as bacc
    import concourse.mybir as mybir
    from concourse import tile

    f32 = mybir.dt.float32
    bf16 = mybir.dt.bfloat16
    Alu = mybir.AluOpType

    nc = bacc.Bacc(None, target_bir_lowering=False)
    inp_d = nc.dram_tensor('inp', [CG, INW], bf16, kind='ExternalInput')
    out_d = nc.dram_tensor('outp', [CG, ONW], f32, kind='ExternalOutput')

    GRP = 2048
    NGRP = NP5 // GRP   # 4

    with tile.TileContext(nc) as tc:
      for _rep in range(n_reps):
        with (
            tc.tile_pool(name='big', bufs=1) as bigp,
            tc.tile_pool(name='oh', bufs=1) as ohp,
            tc.tile_pool(name='tbsb', bufs=2) as tbsbp,
            tc.tile_pool(name='blg', bufs=3) as blgp,
            tc.tile_pool(name='rr', bufs=2) as rrp,
            tc.tile_pool(name='ps_tb', bufs=2, space='PSUM') as ps_tb,
            tc.tile_pool(name='ps_nu', bufs=2, space='PSUM') as ps_nu,
            tc.tile_pool(name='ps_rb', bufs=1, space='PSUM') as ps_rb,
        ):
            inp = bigp.tile([CG, INW], bf16, tag='inp', name='inp')
            nc.sync.dma_start(inp[:], inp_d[:])
            b5 = bigp.tile([CG, NP5], bf16, tag='b5', name='b5')
            osb = bigp.tile([CG, ONW], f32, tag='osb', name='osb')

            def tcol(off, w):
                return inp[:, off:off + w]

            # one-hot builds: compare xs-broadcast block against the two
            # comparator columns (broadcast along free via stride-0 AP)
            xsv = inp[:, OFF_XS:OFF_XS + NP5]
            ohs = []
            for hset in range(2):
                col = inp[:, OFF_SV + hset:OFF_SV + hset + 1]
                svb = bass.AP(col.tensor, col.offset, [list(col.ap[0]), [0, NP5]])
                oh = ohp.tile([CG, NP5], bf16, tag=f'oh{hset}', name=f'oh{hset}')
                nc.vector.tensor_tensor(oh[:], xsv, svb, Alu.is_equal)
                ohs.append(oh)

            # 2 alternating nu bank-tiles; filler rows memset once (the host
            # only reads rows 0..15 of each 32-block, but CoreSim-style
            # uninitialized-read checks want them written)
            nu_banks = [ps_nu.tile([CG, NCHUNK], f32, tag='nu', name=f'nu{b}')
                        for b in range(2)]
            for b in range(2):
                nc.vector.memset(nu_banks[b][:], 1.0)

            b5v = b5[:].rearrange('p (u l) -> p u l', l=K)
            blgs = {}
            r_sbs = {}

            def a_pair(g4, pp):
                po = pp * 2 * NCHUNK
                tb_ps = ps_tb.tile([CG, 2 * NCHUNK], f32, tag='tb', name='tb')
                for s in range(2):
                    co = g4 * GRP + po + s * NCHUNK
                    nc.tensor.matmul(tb_ps[:, s * NCHUNK:(s + 1) * NCHUNK],
                                     tcol(OFF_T6A, CG), ohs[0][:, co:co + NCHUNK],
                                     start=True, stop=False)
                    nc.tensor.matmul(tb_ps[:, s * NCHUNK:(s + 1) * NCHUNK],
                                     tcol(OFF_T6B, CG), ohs[1][:, co:co + NCHUNK],
                                     start=False, stop=True)
                tbsb, blg = blgs[g4]
                nc.vector.tensor_scalar_mul(tbsb[:, po:po + 2 * NCHUNK], tb_ps[:], 1.0)
                nc.vector.tensor_mul(
                    blg[:, po:po + 2 * NCHUNK], tbsb[:, po:po + 2 * NCHUNK],
                    inp[:, OFF_BX + g4 * GRP + po:OFF_BX + g4 * GRP + po + 2 * NCHUNK])

            def emit_nu(g4, cc):
                _, blg = blgs[g4]
                nc.tensor.matmul(nu_banks[g4 % 2][32 * cc:32 * cc + 16, :],
                                 tcol(OFF_SEL, 16),
                                 blg[:, cc * NCHUNK:(cc + 1) * NCHUNK],
                                 start=True, stop=True, tile_position=(0, 32 * cc))

            def emit_recip(g4):
                r_sb = rrp.tile([CG, NCHUNK], bf16, tag='r', name='r')
                with nc.allow_low_precision(reason='bf16 normalizer broadcast; validated vs reference'):
                    nc.vector.reciprocal(r_sb[:], nu_banks[g4 % 2][:])
                # ship raw nu for host-side exact ln()
                nc.vector.tensor_scalar_mul(
                    osb[:, NP4 + g4 * NCHUNK:NP4 + (g4 + 1) * NCHUNK],
                    nu_banks[g4 % 2][:], 1.0)
                r_sbs[g4] = r_sb

            def b_pair(g4, pp):
                r_sb = r_sbs[g4]
                _, blg = blgs[g4]
                rb_ps = ps_rb.tile([CG, 2 * NCHUNK], f32, tag='rb', name='rb')
                for s in range(2):
                    cc = pp * 2 + s
                    poff = 32 * cc
                    nc.tensor.matmul(rb_ps[:, s * NCHUNK:(s + 1) * NCHUNK],
                                     tcol(OFF_E, CG)[poff:poff + 16, :],
                                     r_sb[poff:poff + 16, :],
                                     start=True, stop=True, tile_position=(poff, 0))
                c0 = g4 * GRP + pp * 2 * NCHUNK
                nc.vector.tensor_mul(b5[:, c0:c0 + 2 * NCHUNK],
                                     blg[:, pp * 2 * NCHUNK:(pp + 1) * 2 * NCHUNK],
                                     rb_ps[:])

            NHC = NP4 // 2

            def level1_half(c):
                tb_ps = ps_tb.tile([CG, NHC], f32, tag='tb', name='tb1')
                for l in range(K):
                    nc.tensor.matmul(tb_ps[:], tcol(OFF_W + CG * l, CG),
                                     b5v[:, c * NHC:(c + 1) * NHC, l],
                                     start=(l == 0), stop=(l == K - 1))
                nc.vector.tensor_mul(
                    osb[:, c * NHC:(c + 1) * NHC], tb_ps[:],
                    inp[:, OFF_BX + NP5 + c * NHC:OFF_BX + NP5 + (c + 1) * NHC])

            for g4 in range(NGRP):
                blgs[g4] = (tbsbp.tile([CG, GRP], bf16, tag='tbsb', name='tbsb'),
                            blgp.tile([CG, GRP], bf16, tag='blg', name='blg'))
                if g4 > 0:
                    emit_recip(g4 - 1)
                a_pair(g4, 0)
                emit_nu(g4, 0)
                emit_nu(g4, 1)
                if g4 > 0:
                    b_pair(g4 - 1, 0)
                a_pair(g4, 1)
                emit_nu(g4, 2)
                emit_nu(g4, 3)
                if g4 > 0:
                    b_pair(g4 - 1, 1)
                if g4 == 2:
                    level1_half(0)
            emit_recip(3)
            b_pair(3, 0)
            b_pair(3, 1)
            level1_half(1)
            nc.sync.dma_start(out_d[:], osb[:])
    if not nc.is_finalized():
        nc.finalize()
    return nc


_BASS_CACHE = {}


def _get_bass():
    if 'nc' not in _BASS_CACHE:
        _BASS_CACHE['nc'] = _build_bass()
    return _BASS_CACHE['nc']


def _make_in_maps(x, host):
    BT = host['BT']
    in_maps = []
    for t in range(NTREE):
        base = t * NT
        for q in range(NQ):
            s6 = base + STARTS[6] + q * LEAVES_Q
            xs = x[s6: s6 + LEAVES_Q]
            # 16x-broadcast layout: partition p = l*16+sh, col = parent
            xsb = np.repeat(xs.reshape(NP5, K).T.astype(BF16), 16, axis=0)  # [128, 8192]
            s5 = base + STARTS[5] + q * NP5
            s4 = base + STARTS[4] + q * NP4
            xi = np.concatenate([x[s5: s5 + NP5], x[s4: s4 + NP4]])
            bxh = BT[xi].T.astype(BF16)                                     # [128, 9216]
            inp = np.concatenate([host['tabsv'], bxh, xsb], axis=1)         # [128, 18834]
            in_maps.append({'inp': np.ascontiguousarray(inp)})
    return in_maps


def kernel(**inputs):
    from concourse.bass_utils import run_bass_kernel_spmd

    A = np.asarray(inputs['A']); B = np.asarray(inputs['B'])
    Pi = np.asarray(inputs['Pi']); SP = np.asarray(inputs['SP'])
    x = np.asarray(inputs['x'])

    host = _build_tables(A, B, Pi, SP)
    Mmat, smB, llLeaf = host['Mmat'], host['smB'], host['llLeaf']

    in_maps = _make_in_maps(x, host)
    nc = _get_bass()
    global _LAST_IN_MAPS
    _LAST_IN_MAPS = in_maps
    res = run_bass_kernel_spmd(nc, in_maps, core_ids=list(range(8)))
    results = res.results

    out = np.zeros((NTREE, NGEN), np.float64)
    for t in range(NTREE):
        base = t * NT
        # leaf log-scale term: histogram x log-table (exact)
        xleaf = x[base + STARTS[6]: base + STARTS[6] + K ** DEPTH]
        pos = np.tile(np.arange(K), K ** (DEPTH - 1))
        cnt = np.bincount(pos * MSYM + xleaf, minlength=K * MSYM).astype(np.float64)
        out[t] += cnt @ llLeaf.reshape(K * MSYM, NGEN)
        beta = np.empty((NQ, NP4, C, NGEN), np.float64)
        for q in range(NQ):
            o = np.asarray(results[t * NQ + q]['outp'], np.float64)  # [128, 3072]
            # level 6->5 log-scales from raw nu: bank b rows 32j..32j+16 = nu
            # of chunk 4b+j (g = row % 32)
            nus = o[:, NP4:].reshape(CG, 4, NCHUNK)
            nus = nus.reshape(4, 32, 4, NCHUNK)[:, :16]  # [blk, g, bank, n]
            out[t] += np.log(nus).sum(axis=(0, 2, 3))
            # depth-4: normalize the unnormalized bl4 here
            bl4 = o[:, :NP4].T.reshape(NP4, C, NGEN)
            nu4 = bl4.sum(1)
            out[t] += np.log(nu4).sum(axis=0)
            beta[q] = bl4 / nu4[:, None]
        bcur = beta.reshape(NQ * NP4, C, NGEN)
        for d in (3, 2, 1):
            nd = K ** d
            bch = bcur.reshape(nd, K, C, NGEN)
            tb = np.einsum('uljg,lijg->uig', bch, Mmat)
            sd = base + STARTS[d]
            bl = tb * np.transpose(smB[:, x[sd: sd + nd]], (1, 0, 2))
            nu = bl.sum(1)
            out[t] += np.log(nu).sum(axis=0)
            bcur = bl / nu[:, None]
        tb0 = np.einsum('ljg,lijg->ig', bcur, Mmat)
        bl0 = tb0 * smB[:, x[base]]
        nu0 = bl0.sum(0)
        out[t] += np.log(nu0)
    return out.astype(np.float32)
